# revision 5
# baseline (speedup 1.0000x reference)
"""AssemblyClassifier Trainium2 kernel: 8-way batch-parallel across NeuronCores.

Per core (batch b), x = input_seq[b] : (28, 16384, 8) f32 is pre-cast to bf16
on the host (numerically identical to the previous on-device SWDGE cast-DMA,
but halves the HBM read stream: 7.34MB instead of 14.68MB per core), viewed
flat as [112 partitions = 4*e + t_hi, (t_lo, f)] (contiguous, so full-rate
DMA runs).  The assembly fold G2 = [-scale*Ef | alpha*(1-Ef)].T @ eq_classes
is computed on HOST (tiny matmul) and shipped pre-masked per t_hi group as a
[128, 4*512] bf16 table.  Per 512-t_lo chunk: 3-level bf16 add tree (l1,
s_raw on DVE; l2 + the NaN is_equal mask on the otherwise-idle GpSimd); DVE
does the fused min/mult select; 16 K=112 matmuls accumulate into two 4-bank
PSUM tiles (one per c-half), evacuated by ONE wide f32->bf16 copy each (ACT
for half 0, DVE for half 1) to amortize per-op overhead.  Read DMAs ride the
ACT HWDGE ring (nc.scalar) and output writes ride the SP ring (nc.sync), so
the two streams round-robin on the 16 SDMA engines and overlap.  Output is
written bf16 [c, t]-transposed and upcast/un-transposed on host.
"""
import os
import sys
import types

import numpy as np

_B, _E, _T, _F = 8, 28, 16384, 8
_A, _C = 1024, 256
_HI = 4                 # t_hi groups (partition dim = 4*e + t_hi)
_TL = _T // _HI         # 4096 t_lo per group
_NT = 512               # t_lo chunk
_NCHUNK = _TL // _NT    # 8
_P = _E * _HI           # 112 active partitions

_cache = {}
LAST_RESULTS = None


def _ensure_axon_hooks():
    """The RL image's antenv lacks axon_hooks; shim it so trace=True works."""
    try:
        import antenv.axon_hooks  # noqa: F401
        return
    except Exception:
        pass
    try:
        from trn_agent_boot.trn_boot import _ntff_profile_via_ctypes
        hook = _ntff_profile_via_ctypes('/opt/axon/libaxon_pjrt.so')
    except Exception:
        hook = None
    m = types.ModuleType('antenv.axon_hooks')
    m.get_axon_ntff_profile_hook = lambda: hook
    m.set_axon_ntff_profile_hook = lambda h: None
    sys.modules['antenv.axon_hooks'] = m


def _build():
    import concourse.bass as bass
    import concourse.mybir as mybir
    from concourse import bacc
    from concourse.tile import TileContext

    F32 = mybir.dt.float32
    BF16 = mybir.dt.bfloat16
    ALU = mybir.AluOpType

    nc = bacc.Bacc("TRN2", target_bir_lowering=False)
    # pre-cast bf16 input: xh[p, (tl f)], p = 4e + t_hi
    xh = nc.declare_dram_parameter("xh", [_P, _TL * _F], BF16, isOutput=False)
    g2 = nc.declare_dram_parameter("g2", [128, _HI * 512], BF16,
                                   isOutput=False)
    # transposed output: out[c, q*2048 + g*512 + tl]; host un-transposes.
    out = nc.declare_dram_parameter("out", [_C, _T], BF16, isOutput=True)

    xhv = xh[:]
    ov = out[:].rearrange("(h c) (q gtl) -> q h c gtl", h=2, q=_NCHUNK)
    ovw = out[:].rearrange("(h c) (q g w tl) -> q w h c g tl", h=2,
                           q=_NCHUNK, g=_HI, w=2)

    with TileContext(nc) as tc:
        with (
            tc.tile_pool(name="const", bufs=1) as constp,
            tc.tile_pool(name="xin", bufs=4) as xin,
            tc.tile_pool(name="work", bufs=2) as work,
            tc.tile_pool(name="mm", bufs=2) as mmp,
            tc.tile_pool(name="psum", bufs=1, space="PSUM") as psp,
            tc.tile_pool(name="outp", bufs=6) as outp,
        ):
            # host-folded, pre-masked tables: [p, g*512 + (0:256 | 256:512)]
            # = (-scale*Ef.T@eq | alpha*(1-Ef).T@eq)[e] iff p == 4e+g else 0
            g2_sb = constp.tile([128, _HI * 512], BF16)
            nc.scalar.dma_start(out=g2_sb[:], in_=g2[:])

            def process(xt, col0, nt, q, w):
                """tree + select + matmuls + evac + out-DMA for nt t_lo cols
                of xt starting at t_lo col0 (chunk q, half w for nt=256)."""
                x3 = xt[0:_P, col0 * _F:(col0 + nt) * _F].rearrange(
                    "p (tl f) -> p tl f", f=_F)
                # 3-level bf16 add tree; l2 + is_eq run on GpSimd to share
                # the elementwise load (all SBUF-only, GpSimd has no PSUM)
                l1 = work.tile([128, nt * 4], BF16, name="l1")
                nc.vector.tensor_tensor(out=l1[0:_P, 0:nt * 4],
                                        in0=x3[:, :, 0:4], in1=x3[:, :, 4:8],
                                        op=ALU.add)
                l13 = l1[0:_P, 0:nt * 4].rearrange("p (tl f) -> p tl f", f=4)
                l2 = work.tile([128, nt * 2], BF16, name="l2")
                nc.gpsimd.tensor_tensor(out=l2[0:_P, 0:nt * 2],
                                        in0=l13[:, :, 0:2], in1=l13[:, :, 2:4],
                                        op=ALU.add)
                l23 = l2[0:_P, 0:nt * 2].rearrange("p (tl f) -> p tl f", f=2)
                s_raw = work.tile([128, nt], BF16, name="s_raw")
                nc.vector.tensor_tensor(out=s_raw[0:_P, 0:nt],
                                        in0=l23[:, :, 0:1], in1=l23[:, :, 1:2],
                                        op=ALU.add)

                obsf = mmp.tile([128, nt], BF16, name="obsf")
                nc.vector.tensor_tensor(out=obsf[0:_P, 0:nt],
                                        in0=s_raw[0:_P, 0:nt],
                                        in1=s_raw[0:_P, 0:nt], op=ALU.is_equal)
                s0 = mmp.tile([128, nt], BF16, name="s0")
                nc.vector.scalar_tensor_tensor(out=s0[0:_P, 0:nt],
                                               in0=s_raw[0:_P, 0:nt],
                                               scalar=3.0e38,
                                               in1=obsf[0:_P, 0:nt],
                                               op0=ALU.min, op1=ALU.mult)

                # out[c, t] matmuls into one 4-bank PSUM tile per c-half,
                # then ONE wide evac copy per half (amortizes op overhead)
                for h in range(2):
                    pt = psp.tile([128, _HI * nt], F32, name=f"pt{h}")
                    for g in range(_HI):
                        dst = pt[:, g * nt:(g + 1) * nt]
                        nc.tensor.matmul(dst,
                                         g2_sb[0:_P, g * 512 + h * 128:
                                               g * 512 + h * 128 + 128],
                                         s0[0:_P, 0:nt],
                                         start=True, stop=False)
                        nc.tensor.matmul(dst,
                                         g2_sb[0:_P, g * 512 + _C + h * 128:
                                               g * 512 + _C + h * 128 + 128],
                                         obsf[0:_P, 0:nt],
                                         start=False, stop=True)
                    og = outp.tile([128, _HI * nt], BF16, name=f"og{h}")
                    if h == 0:
                        nc.scalar.copy(out=og[:, :], in_=pt[:, :])
                    else:
                        nc.vector.tensor_copy(out=og[:, :], in_=pt[:, :])
                    if nt == _NT:
                        nc.sync.dma_start(out=ov[q, h], in_=og[:, :])
                    else:
                        nc.sync.dma_start(out=ovw[q, w, h],
                                          in_=og[:, :].rearrange(
                                              "c (g tl) -> c g tl", g=_HI))

            for ci in range(_NCHUNK):
                # one read per 512-t_lo chunk: [112, 8KB] = 917KB on the ACT
                # HWDGE ring (separate from the SP write ring, so the read
                # and write streams round-robin on the 16 SDMA engines)
                c0 = ci * _NT * _F
                xt = xin.tile([128, _NT * _F], BF16, name="xt")
                with tc.high_priority():
                    nc.scalar.dma_start(out=xt[0:_P, :],
                                        in_=xhv[:, c0:c0 + _NT * _F])
                if ci < _NCHUNK - 1:
                    process(xt, 0, _NT, ci, 0)
                else:
                    # split the final chunk to shorten the pipeline drain
                    process(xt, 0, 256, ci, 0)
                    process(xt, 256, 256, ci, 1)
    nc.compile()
    return nc


def _get_nc():
    if "nc" not in _cache:
        _ensure_axon_hooks()
        from concourse import bass_utils
        bass_utils.upload_artifacts = lambda tmpdir: "local://skipped"
        _cache["nc"] = _build()
    return _cache["nc"]


def kernel(input_seq, eq_classes, scale, alpha, edge_present):
    global LAST_RESULTS
    import ml_dtypes
    x = np.asarray(input_seq, dtype=np.float32)
    eqc = np.asarray(eq_classes, dtype=np.float32)
    ef = np.asarray(edge_present).astype(np.float32)
    sc = float(np.asarray(scale))
    al = float(np.asarray(alpha))

    # host-side fold of the assembly axis (tiny matmul), pre-masked per t_hi
    # group
    g_edge = (ef.T @ eqc) * (-sc)              # (E, C)
    g_no = ((1.0 - ef).T @ eqc) * al           # (E, C)
    g2 = np.zeros((128, _HI * 512), np.float32)
    for e in range(_E):
        for g in range(_HI):
            p = _HI * e + g
            g2[p, g * 512:g * 512 + _C] = g_edge[e]
            g2[p, g * 512 + _C:(g + 1) * 512] = g_no[e]
    g2 = g2.astype(ml_dtypes.bfloat16)

    # bf16 cast; layout [b, (e hi), (tl f)] is just a reshape of (B,E,T,F)
    xh_all = x.reshape(_B, _P, _TL * _F).astype(ml_dtypes.bfloat16)

    nc = _get_nc()
    from concourse import bass_utils
    in_maps = [{"xh": xh_all[b], "g2": g2} for b in range(_B)]
    trace = bool(os.environ.get("KERNEL_TRACE"))
    res = bass_utils.run_bass_kernel_spmd(nc, in_maps, core_ids=list(range(_B)),
                                          trace=trace)
    LAST_RESULTS = res
    outs = []
    for b in range(_B):
        # stored [c, q, g, tl]; logical t = g*4096 + q*512 + tl
        a = np.asarray(res.results[b]["out"]).reshape(_C, _NCHUNK, _HI, _NT)
        outs.append(a.transpose(2, 1, 3, 0).reshape(_T, _C).astype(np.float32))
    return np.stack(outs, axis=0)


# revision 6
# speedup vs baseline: 1.1984x; 1.1984x over previous
"""AssemblyClassifier Trainium2 kernel: 8-way batch-parallel across NeuronCores.

Per core (batch b), x = input_seq[b] : (28, 16384, 8) f32 is pre-cast to bf16
on the host (numerically identical to the previous on-device SWDGE cast-DMA,
but halves the HBM read stream: 7.34MB instead of 14.68MB per core) and laid
out [112 partitions = 4*e + t_hi, (chunk q, f, t_lo)] — f-major WITHIN each
512-t_lo chunk, so every level of the 3-level feature add tree is a
unit-stride block add (in0/in1 both contiguous) and DVE runs them in its 2x
bf16 packed mode.  The assembly fold G2 = [-scale*Ef | alpha*(1-Ef)].T @
eq_classes is computed on HOST (tiny matmul) and shipped pre-masked per t_hi
group as a [128, 4*512] bf16 table (loaded on the write ring, which is idle
at start).  Per 512-t_lo chunk: DVE tree + NaN is_equal mask + fused
min/mult select; 16 K=112 matmuls accumulate into two 4-bank PSUM tiles (one
per c-half), evacuated by wide f32->bf16 copies (ACT takes ~80%, DVE the
rest, balancing engine busy).  Read DMAs ride the GpSimd SWDGE ring and
output writes ride the SP HWDGE ring (nc.sync), so the two streams
round-robin on the 16 SDMA engines and overlap.  Output is written bf16
[c, t]-transposed and upcast/un-transposed on host.
"""
import os
import sys
import types

import numpy as np

_B, _E, _T, _F = 8, 28, 16384, 8
_A, _C = 1024, 256
_HI = 4                 # t_hi groups (partition dim = 4*e + t_hi)
_TL = _T // _HI         # 4096 t_lo per group
_NT = 512               # t_lo chunk
_NCHUNK = _TL // _NT    # 8
_P = _E * _HI           # 112 active partitions
_CW = _NT * _F          # 4096 columns per chunk block

_cache = {}
LAST_RESULTS = None


def _ensure_axon_hooks():
    """The RL image's antenv lacks axon_hooks; shim it so trace=True works."""
    try:
        import antenv.axon_hooks  # noqa: F401
        return
    except Exception:
        pass
    try:
        from trn_agent_boot.trn_boot import _ntff_profile_via_ctypes
        hook = _ntff_profile_via_ctypes('/opt/axon/libaxon_pjrt.so')
    except Exception:
        hook = None
    m = types.ModuleType('antenv.axon_hooks')
    m.get_axon_ntff_profile_hook = lambda: hook
    m.set_axon_ntff_profile_hook = lambda h: None
    sys.modules['antenv.axon_hooks'] = m


def _build():
    import concourse.bass as bass
    import concourse.mybir as mybir
    from concourse import bacc
    from concourse.tile import TileContext

    F32 = mybir.dt.float32
    BF16 = mybir.dt.bfloat16
    ALU = mybir.AluOpType

    nc = bacc.Bacc("TRN2", target_bir_lowering=False)
    # pre-cast bf16 input: xh[p, (q f tl)], p = 4e + t_hi
    xh = nc.declare_dram_parameter("xh", [_P, _TL * _F], BF16, isOutput=False)
    g2 = nc.declare_dram_parameter("g2", [128, _HI * 512], BF16,
                                   isOutput=False)
    # transposed output: out[c, q*2048 + g*512 + tl]; host un-transposes.
    out = nc.declare_dram_parameter("out", [_C, _T], BF16, isOutput=True)

    xhv = xh[:]
    ov = out[:].rearrange("(h c) (q gtl) -> q h c gtl", h=2, q=_NCHUNK)

    with TileContext(nc) as tc:
        with (
            tc.tile_pool(name="const", bufs=1) as constp,
            tc.tile_pool(name="xin", bufs=3) as xin,
            tc.tile_pool(name="work", bufs=2) as work,
            tc.tile_pool(name="mm", bufs=2) as mmp,
            tc.tile_pool(name="psum", bufs=1, space="PSUM") as psp,
            tc.tile_pool(name="outp", bufs=4) as outp,
        ):
            # host-folded, pre-masked tables: [p, g*512 + (0:256 | 256:512)]
            # = (-scale*Ef.T@eq | alpha*(1-Ef).T@eq)[e] iff p == 4e+g else 0.
            # Rides the write ring, which is otherwise idle at start.
            g2_sb = constp.tile([128, _HI * 512], BF16)
            nc.sync.dma_start(out=g2_sb[:], in_=g2[:])

            def process(xt, col0, q):
                """tree + select + matmuls + evac + out-DMA for the chunk
                block at xt cols [col0, col0+4096) (chunk q)."""
                nt = _NT
                xc = xt[0:_P, col0:col0 + _CW]
                # f-major block layout -> all tree levels are unit-stride
                # contiguous adds (DVE 2x bf16 packed mode)
                l1 = work.tile([128, nt * 4], BF16, name="l1")
                nc.vector.tensor_tensor(out=l1[0:_P, :],
                                        in0=xc[:, 0:nt * 4],
                                        in1=xc[:, nt * 4:nt * 8], op=ALU.add)
                l2 = work.tile([128, nt * 2], BF16, name="l2")
                nc.vector.tensor_tensor(out=l2[0:_P, :],
                                        in0=l1[0:_P, 0:nt * 2],
                                        in1=l1[0:_P, nt * 2:nt * 4],
                                        op=ALU.add)
                s_raw = work.tile([128, nt], BF16, name="s_raw")
                nc.vector.tensor_tensor(out=s_raw[0:_P, :],
                                        in0=l2[0:_P, 0:nt],
                                        in1=l2[0:_P, nt:nt * 2], op=ALU.add)

                obsf = mmp.tile([128, nt], BF16, name="obsf")
                nc.vector.tensor_tensor(out=obsf[0:_P, :],
                                        in0=s_raw[0:_P, :],
                                        in1=s_raw[0:_P, :], op=ALU.is_equal)
                s0 = mmp.tile([128, nt], BF16, name="s0")
                nc.vector.scalar_tensor_tensor(out=s0[0:_P, :],
                                               in0=s_raw[0:_P, :],
                                               scalar=3.0e38,
                                               in1=obsf[0:_P, :],
                                               op0=ALU.min, op1=ALU.mult)

                # out[c, t] matmuls into one 4-bank PSUM tile per c-half,
                # then wide evac copies (ACT ~80% / DVE ~20% of columns)
                ogs = []
                for h in range(2):
                    pt = psp.tile([128, _HI * nt], F32, name=f"pt{h}")
                    for g in range(_HI):
                        dst = pt[:, g * nt:(g + 1) * nt]
                        nc.tensor.matmul(dst,
                                         g2_sb[0:_P, g * 512 + h * 128:
                                               g * 512 + h * 128 + 128],
                                         s0[0:_P, :],
                                         start=True, stop=False)
                        nc.tensor.matmul(dst,
                                         g2_sb[0:_P, g * 512 + _C + h * 128:
                                               g * 512 + _C + h * 128 + 128],
                                         obsf[0:_P, :],
                                         start=False, stop=True)
                    og = outp.tile([128, _HI * nt], BF16, name=f"og{h}")
                    ogs.append((pt, og))
                (pt0, og0), (pt1, og1) = ogs
                cut = 1280
                nc.scalar.copy(out=og0[:, :], in_=pt0[:, :])
                nc.scalar.copy(out=og1[:, 0:cut], in_=pt1[:, 0:cut])
                nc.vector.tensor_copy(out=og1[:, cut:], in_=pt1[:, cut:])
                for h, og in ((0, og0), (1, og1)):
                    nc.sync.dma_start(out=ov[q, h], in_=og[:, :])

            for ri in range(_NCHUNK // 2):
                # one SWDGE read per TWO chunk blocks: [112, 16KB] = 1.83MB
                c0 = ri * 2 * _CW
                xt = xin.tile([128, 2 * _CW], BF16, name="xt")
                with tc.high_priority():
                    nc.gpsimd.dma_start(out=xt[0:_P, :],
                                        in_=xhv[:, c0:c0 + 2 * _CW])
                for sub in range(2):
                    process(xt, sub * _CW, 2 * ri + sub)
    nc.compile()
    return nc


def _get_nc():
    if "nc" not in _cache:
        _ensure_axon_hooks()
        from concourse import bass_utils
        bass_utils.upload_artifacts = lambda tmpdir: "local://skipped"
        _cache["nc"] = _build()
    return _cache["nc"]


def kernel(input_seq, eq_classes, scale, alpha, edge_present):
    global LAST_RESULTS
    import ml_dtypes
    x = np.asarray(input_seq, dtype=np.float32)
    eqc = np.asarray(eq_classes, dtype=np.float32)
    ef = np.asarray(edge_present).astype(np.float32)
    sc = float(np.asarray(scale))
    al = float(np.asarray(alpha))

    # host-side fold of the assembly axis (tiny matmul), pre-masked per t_hi
    # group
    g_edge = (ef.T @ eqc) * (-sc)              # (E, C)
    g_no = ((1.0 - ef).T @ eqc) * al           # (E, C)
    g2 = np.zeros((128, _HI * 512), np.float32)
    for e in range(_E):
        for g in range(_HI):
            p = _HI * e + g
            g2[p, g * 512:g * 512 + _C] = g_edge[e]
            g2[p, g * 512 + _C:(g + 1) * 512] = g_no[e]
    g2 = g2.astype(ml_dtypes.bfloat16)

    # bf16 cast + f-major-per-chunk layout:
    # xh[b, 4e+hi, q*4096 + f*512 + tl] = x[b, e, hi*4096 + q*512 + tl, f]
    xq = x.reshape(_B, _E, _HI, _NCHUNK, _NT, _F).astype(ml_dtypes.bfloat16)
    xh_all = np.ascontiguousarray(xq.transpose(0, 1, 2, 3, 5, 4)).reshape(
        _B, _P, _TL * _F)

    nc = _get_nc()
    from concourse import bass_utils
    in_maps = [{"xh": xh_all[b], "g2": g2} for b in range(_B)]
    trace = bool(os.environ.get("KERNEL_TRACE"))
    res = bass_utils.run_bass_kernel_spmd(nc, in_maps, core_ids=list(range(_B)),
                                          trace=trace)
    LAST_RESULTS = res
    outs = []
    for b in range(_B):
        # stored [c, q, g, tl]; logical t = g*4096 + q*512 + tl
        a = np.asarray(res.results[b]["out"]).reshape(_C, _NCHUNK, _HI, _NT)
        outs.append(a.transpose(2, 1, 3, 0).reshape(_T, _C).astype(np.float32))
    return np.stack(outs, axis=0)
